# revision 20
# baseline (speedup 1.0000x reference)
"""Trainium2 Bass kernel: FADEv4 retrieval-kNN head (nn_FADEv4_7026566496861).

Math (per image n):
    cls  = l2norm(mean_s(x_support_cls[n]))          # [1,D]
    q    = l2norm(x_query[n])                        # [Tq,D]
    s    = l2norm(x_support[n])                      # [Ts,D]
    sim  = q @ s.T                                   # [Tq,Ts]
    dmin = 1 - max_ts(sim); idx = argmax_ts(sim)
    pred = sigmoid(q@W1 + s[idx]@W2 + cls@W3 + b)
    out0 = (pred*dmin).reshape(N,1,37,37); out1 = pred.reshape(N,1,37,37)

Sharding: data-parallel over N=16 images -> 8 cores x 2 images, no collectives.

v4 design:
  * sT ([Ts,D] -> [D,Ts]) transposes run on the DMA XBAR (bf16) from the
    normalized SBUF tile straight into the [128, KC, 512] matmul layout.
    Loads and XBAR transposes are emitted in separate phases (loads run
    2-3 j-chunks ahead) so a transpose instruction never sits in the sync
    DMA FIFO with an unmet dependency in front of later input loads
    (head-of-line blocking -- this collapsed the producer pipeline in v3).
  * qT transposes stay on the PE (fused normalize-transpose), interleaved
    with the first j-chunk's matmuls; image 1's qT chunks are emitted
    inside image 0's j-loop.
  * Consumer-side small DMAs (p2 staging, cls broadcast, outputs) go
    through the GPSIMD SWDGE queue, keeping the sync HWDGE queue a pure
    producer stream.
  * W1 rides as an extra support column (sim[:,Ts] = q@W1) and W2 as an
    extra query column (sim[Tq,:] = s@W2) of the big matmul.
  * max/argmax on DVE per [mreal,512] PSUM chunk (max8 + find_index8);
    chunk results combined per m-block with the match_replace one-hot
    trick (first-occurrence argmax semantics).
"""

import os
from contextlib import ExitStack

import numpy as np

import concourse.bass as bass
import concourse.mybir as mybir
import concourse.tile as tile
from concourse import bacc, bass_isa
from concourse.bass import ds, ts, IndirectOffsetOnAxis
from concourse.bass_utils import run_bass_kernel_spmd
from concourse.masks import make_identity

F32 = mybir.dt.float32
BF16 = mybir.dt.bfloat16
U32 = mybir.dt.uint32
AX = mybir.AxisListType
OP = mybir.AluOpType
ACTF = mybir.ActivationFunctionType

N_FULL, TQ, TS, S, D = 16, 1369, 5476, 4, 768
SIDE = 37
KC = D // 128            # 6 contraction chunks
W2COL = 1376             # W2 column (inside zero-padded tail of last m-block)
QCOLS = 1408             # 11 x 128 transpose-written query columns
MB = 11                  # m-blocks; last reads cols 1280..1377 (97 cols)
NB = 11                  # 512-wide support chunks (last: 357 incl W1)
NEG = -1.0e30

N_CORES = 8
PER_CORE = N_FULL // N_CORES

MM_DTYPE = BF16


def _emit_consts(nc, const_pool, scratch, w_head, b_head):
    ident_mm = const_pool.tile([128, 128], MM_DTYPE)
    make_identity(nc, ident_mm[:, :])
    c512u = scratch.tile([128, MB, NB * 8], U32, tag="c512u", bufs=1)
    c512f = const_pool.tile([128, MB, NB * 8], F32)
    nc.gpsimd.iota(c512u[:, :, :], pattern=[[0, MB], [512, NB], [0, 8]], base=0,
                   channel_multiplier=0)
    nc.vector.tensor_copy(c512f[:, :, :], c512u[:, :, :])
    w1s = const_pool.tile([128, KC], F32)
    w1b = const_pool.tile([128, KC], BF16)
    w2f = const_pool.tile([128, KC], F32)
    w2s = const_pool.tile([128, KC], BF16)
    w3 = const_pool.tile([1, D], F32)
    bh = const_pool.tile([1, 1], F32)
    for k in range(KC):
        nc.sync.dma_start(out=w1s[:, k:k + 1], in_=w_head[ds(128 * k, 128), :])
        nc.sync.dma_start(out=w2f[:, k:k + 1], in_=w_head[ds(D + 128 * k, 128), :])
    nc.vector.tensor_copy(w1b[:, :], w1s[:, :])
    nc.vector.tensor_copy(w2s[:, :], w2f[:, :])
    nc.sync.dma_start(out=w3[0:1, :], in_=w_head[ds(2 * D, D), :])
    nc.sync.dma_start(out=bh[:, :], in_=b_head[:, :])
    return ident_mm, c512f, w1b, w2s, w3, bh


class Image:
    """Per-image SBUF state (allocated from a bufs=2 pool -> double buffered)."""

    def __init__(self, nc, img_pool, n):
        self.n = n
        self.qT = img_pool.tile([128, KC, QCOLS], MM_DTYPE, tag="qT")
        self.Mc8 = img_pool.tile([128, MB, NB, 8], F32, tag="Mc8")
        self.Ic8 = img_pool.tile([128, MB, NB, 8], U32, tag="Ic8")
        self.p1 = img_pool.tile([128, MB], F32, tag="p1")
        self.p2g = img_pool.tile([128, MB], F32, tag="p2g")
        self.gidx = img_pool.tile([128, MB], U32, tag="gidx")
        self.gmax = img_pool.tile([128, MB], F32, tag="gmax")
        self.icf = img_pool.tile([128, MB, NB * 8], F32, tag="icf", bufs=1)
        self.dmin = img_pool.tile([128, MB], F32, tag="dmin")
        self.c3b = img_pool.tile([128, 1], F32, tag="c3b")
        self.pred = img_pool.tile([128, MB], F32, tag="predb")
        self.o0 = img_pool.tile([128, MB], F32, tag="o0b")
        # per-chunk raw/normalized staging, queries + supports
        self.qraw = {}
        self.qnm = {}
        self.sraw = {}
        self.snm = {}
        nc.vector.memset(self.gidx[:, :], 0)


def _emit_cls(nc, st, scratch, consts, x_cls, c3d):
    """cls head scalar: c3b = (sum_cls . W3)/||sum_cls|| + b, broadcast."""
    (ident_mm, c512f, w1b, w2s, w3, bh) = consts
    n = st.n
    clsbig = scratch.tile([1, S * D], F32, tag="clsbig", bufs=1)
    nc.gpsimd.dma_start(out=clsbig[:, :], in_=x_cls[n])
    clsum = scratch.tile([1, D], F32, tag="clsum")
    nc.vector.tensor_add(clsum[:, :], clsbig[:, 0:D], clsbig[:, D:2 * D])
    nc.vector.tensor_add(clsum[:, :], clsum[:, :], clsbig[:, 2 * D:3 * D])
    nc.vector.tensor_add(clsum[:, :], clsum[:, :], clsbig[:, 3 * D:4 * D])
    sc3 = scratch.tile([1, D], F32, tag="sc3")
    ss3 = scratch.tile([1, 8], F32, tag="ss3")
    nc.vector.tensor_mul(sc3[:, :], clsum[0:1, :], clsum[0:1, :])
    nc.vector.tensor_reduce(out=ss3[:, 0:1], in_=sc3[:, :], axis=AX.X, op=OP.add)
    nc.vector.tensor_mul(sc3[:, :], clsum[0:1, :], w3[:, :])
    nc.vector.tensor_reduce(out=ss3[:, 1:2], in_=sc3[:, :], axis=AX.X, op=OP.add)
    nc.scalar.sqrt(ss3[:, 2:3], ss3[:, 0:1])
    nc.vector.reciprocal(ss3[:, 3:4], ss3[:, 2:3])
    nc.vector.tensor_mul(ss3[:, 4:5], ss3[:, 1:2], ss3[:, 3:4])
    nc.vector.tensor_add(ss3[:, 5:6], ss3[:, 4:5], bh[:, 0:1])
    nc.gpsimd.dma_start(out=c3d[:, :], in_=ss3[0:1, 5:6])
    nc.gpsimd.dma_start(out=st.c3b[:, :], in_=c3d[:, :].to_broadcast((128, 1)))


RAW_BUFS = {"q_raw": 6, "s_raw": 12}
NM_BUFS = {"q_nm": 3, "s_nm": 8}


def _emit_load(nc, scratch, store, key, src_row0, tok0, rows, tag):
    raw = scratch.tile([128, D], F32, tag=tag, bufs=RAW_BUFS[tag])
    nc.sync.dma_start(out=raw[:rows, :], in_=src_row0[ds(tok0, rows), :])
    store[key] = raw


def _emit_norm(nc, scratch, store_raw, store_nm, key, rows, zero_pad, tag,
               dve_square=False):
    """Normalize the loaded 128-row chunk into a bf16 tile.

    The square pass writes its (unused) main output into the nm tile that
    the final normalize-mul overwrites -- only the accumulator matters.
    dve_square moves that pass to the DVE (used for query chunks, whose
    normalize competes with support chunks for the scalar engine).
    """
    raw = store_raw.pop(key)
    ssn = scratch.tile([128, 4], F32, tag="nt_ss", bufs=12)
    s_nm = scratch.tile([128, D], MM_DTYPE, tag=tag, bufs=NM_BUFS[tag])
    if dve_square:
        nc.vector.tensor_tensor(
            s_nm[:rows, :], raw[:rows, :], raw[:rows, :], OP.mult
        )
        nc.vector.tensor_reduce(
            out=ssn[:rows, 0:1], in_=s_nm[:rows, :], axis=AX.X, op=OP.add
        )
    else:
        nc.scalar.activation(
            s_nm[:rows, :], raw[:rows, :], ACTF.Square, accum_out=ssn[:rows, 0:1]
        )
    nc.scalar.sqrt(ssn[:rows, 1:2], ssn[:rows, 0:1])
    nc.vector.reciprocal(ssn[:rows, 2:3], ssn[:rows, 1:2])
    if zero_pad and rows < 128:
        base = (rows // 32) * 32
        nc.vector.memset(s_nm[base:128, :], 0)
    nc.scalar.mul(s_nm[:rows, :], raw[:rows, :], ssn[:rows, 2:3])
    store_nm[key] = (s_nm, rows)


def _emit_qT_load(nc, st, scratch, x_query, c):
    tok0 = c * 128
    rows = min(128, TQ - tok0)
    _emit_load(nc, scratch, st.qraw, c, x_query[st.n], tok0, rows, tag="q_raw")


def _emit_qT_norm(nc, st, scratch, c, dve_square=True):
    rows = min(128, TQ - c * 128)
    _emit_norm(nc, scratch, st.qraw, st.qnm, c, rows, zero_pad=False, tag="q_nm",
               dve_square=dve_square)


def _emit_qT_xpose(nc, st, psum_t, consts, c):
    """PE transpose one normalized query chunk into qT."""
    (ident_mm, c512f, w1b, w2s, w3, bh) = consts
    rows = min(128, TQ - c * 128)
    s_nm, _ = st.qnm.pop(c)
    for g in range(KC // 3):
        pst = psum_t.tile([128, 3, 128], MM_DTYPE, tag="nt_ps")
        for kk in range(3):
            k = 3 * g + kk
            nc.tensor.transpose(
                pst[:, kk, :rows], s_nm[:rows, ts(k, 128)], ident_mm[:rows, :rows]
            )
        nc.vector.tensor_copy(
            st.qT[:, 3 * g:3 * g + 3, ds(c * 128, rows)], pst[:, :, :rows]
        )
    if c == MB - 1:
        # last chunk: zero cols [1369..1408) then drop W2 into col 1376
        nc.vector.memset(st.qT[:, :, TQ:QCOLS], 0)
        for k in range(KC):
            nc.vector.tensor_copy(st.qT[:, k, W2COL:W2COL + 1], w2s[:, k:k + 1])


def _sub_chunks(j):
    nreal = 512 if j < NB - 1 else TS - 512 * (NB - 1)
    out = []
    off = 0
    while off < nreal:
        rows = min(128, nreal - off)
        out.append((off, rows))
        off += rows
    return out


def _emit_sT_load(nc, st, scratch, x_support, j):
    for (off, rows) in _sub_chunks(j):
        _emit_load(nc, scratch, st.sraw, (j, off), x_support[st.n],
                   512 * j + off, rows, tag="s_raw")


def _emit_sT_norm(nc, st, scratch, j, dve_square=False):
    for (off, rows) in _sub_chunks(j):
        _emit_norm(nc, scratch, st.sraw, st.snm, (j, off), rows,
                   zero_pad=True, tag="s_nm", dve_square=dve_square)


def _emit_sT_xpose(nc, st, spool, psum_t, consts, j):
    """Transpose one normalized 512-wide support chunk into matmul layout.

    Sub-chunks 0-1 ride the DMA XBAR; the rest go through PE identity
    transposes with DVE copy-back, splitting the load between the two
    near-saturated paths (DMA serial time vs PE time).
    """
    (ident_mm, c512f, w1b, w2s, w3, bh) = consts
    sT = spool.tile([128, KC, 512], MM_DTYPE, tag="sT")
    for si, (off, rows) in enumerate(_sub_chunks(j)):
        s_nm, _ = st.snm.pop((j, off))
        if si < 2:
            nc.sync.dma_start(
                out=sT[:, :, ds(off, 128)], in_=s_nm[:, :], transpose=True
            )
        else:
            for g in range(KC // 3):
                pst = psum_t.tile([128, 3, 128], MM_DTYPE, tag="nt_ps")
                for kk in range(3):
                    k = 3 * g + kk
                    nc.tensor.transpose(
                        pst[:, kk, :rows], s_nm[:rows, ts(k, 128)],
                        ident_mm[:rows, :rows],
                    )
                if g == 0:
                    nc.vector.tensor_copy(
                        sT[:, 0:3, ds(off, rows)], pst[:, :, :rows]
                    )
                else:
                    nc.scalar.copy(
                        sT[:, 3:6, ds(off, rows)], pst[:, :, :rows]
                    )
    if j == NB - 1:
        nreal = TS - 512 * (NB - 1)
        for k in range(KC):
            nc.vector.tensor_copy(sT[:, k, nreal:nreal + 1], w1b[:, k:k + 1])
    return sT


def _emit_mm_block(nc, st, psum_mm, scratch, sT, j, m, p2d):
    """Matmuls + max/argmax for one (j-chunk, m-block)."""
    ncols = 512 if j < NB - 1 else (TS - 512 * (NB - 1)) + 1  # incl W1 col
    nreal = ncols if j < NB - 1 else ncols - 1
    mcols = 128 if m < MB - 1 else 97
    mreal = 128 if m < MB - 1 else TQ - 128 * (MB - 1)        # 89 on last
    bp = psum_mm.tile([128, 512], F32, tag="bigps")
    for k in range(KC):
        nc.tensor.matmul(
            bp[:mcols, :ncols],
            lhsT=st.qT[:, k, ds(m * 128, mcols)],
            rhs=sT[:, k, :ncols],
            start=(k == 0), stop=(k == KC - 1),
        )
    nc.vector.max(st.Mc8[:mreal, m, j, :], bp[:mreal, :nreal])
    nc.vector.max_index(
        st.Ic8[:mreal, m, j, :], st.Mc8[:mreal, m, j, :], bp[:mreal, :nreal]
    )
    if j == NB - 1:
        nc.vector.tensor_copy(st.p1[:mreal, m:m + 1], bp[:mreal, nreal:nreal + 1])
    if m == MB - 1:
        p2c = scratch.tile([128, 512], F32, tag="p2c", bufs=1)
        nc.vector.tensor_copy(p2c[96:97, :nreal], bp[96:97, :nreal])
        nc.gpsimd.dma_start(out=p2d[ds(512 * j, nreal), 0], in_=p2c[96:97, :nreal])


BIGIDX = 1.0e30


def _emit_icf_m(nc, st, consts, m):
    """icf[m] = float(Ic8[m]) + chunk offsets."""
    (ident_mm, c512f, w1b, w2s, w3, bh) = consts
    mreal = 128 if m < MB - 1 else TQ - 128 * (MB - 1)
    nc.vector.tensor_copy(
        st.icf[:mreal, m, :],
        st.Ic8[:mreal, m, :, :].rearrange("p a b -> p (a b)"),
    )
    nc.vector.tensor_add(
        st.icf[:mreal, m, :], st.icf[:mreal, m, :], c512f[:mreal, 0, :]
    )


def _emit_argmax_m(nc, st, scratch, consts, m):
    """Global argmax over chunk partials for one m-block (3 wide DVE ops)."""
    mreal = 128 if m < MB - 1 else TQ - 128 * (MB - 1)
    W = NB * 8
    mc = st.Mc8[:mreal, m, :, :].rearrange("p a b -> p (a b)")
    gm8 = scratch.tile([128, 8], F32, tag="gm8")
    nc.vector.max(gm8[:mreal, :], mc)
    nc.vector.tensor_copy(st.gmax[:mreal, m:m + 1], gm8[:mreal, 0:1])
    # masked = (mc != gmax)*BIGIDX + icf ; gidx = min(masked) -> first occurrence
    scr = scratch.tile([128, W], F32, tag="scr")
    nc.vector.tensor_scalar(
        out=scr[:mreal, :], in0=mc, scalar1=gm8[:mreal, 0:1], scalar2=BIGIDX,
        op0=OP.not_equal, op1=OP.mult,
    )
    scr2 = scratch.tile([128, W], F32, tag="scr2")
    gidxf = scratch.tile([128, 1], F32, tag="gidxf")
    nc.vector.tensor_tensor(
        scr2[:mreal, :], scr[:mreal, :], st.icf[:mreal, m, :], OP.add
    )
    nc.vector.tensor_reduce(
        out=gidxf[:mreal, :], in_=scr2[:mreal, :], axis=AX.X, op=OP.min
    )
    nc.vector.tensor_copy(st.gidx[:mreal, m:m + 1], gidxf[:mreal, :])
    nc.scalar.activation(
        st.dmin[:mreal, m:m + 1], gm8[:mreal, 0:1], ACTF.Copy,
        bias=1.0, scale=-1.0,
    )


def _emit_head_m(nc, st, scratch, m, p2d):
    """Gather p2 = s[idx]@W2 and apply the sigmoid head for one m-block."""
    mreal = 128 if m < MB - 1 else TQ - 128 * (MB - 1)
    nc.gpsimd.indirect_dma_start(
        out=st.p2g[:, m:m + 1], out_offset=None, in_=p2d[:, :],
        in_offset=IndirectOffsetOnAxis(ap=st.gidx[:, m:m + 1], axis=0),
    )
    lg = scratch.tile([128, 1], F32, tag="lg")
    nc.vector.tensor_add(lg[:mreal, :], st.p1[:mreal, m:m + 1], st.p2g[:mreal, m:m + 1])
    nc.scalar.activation(
        st.pred[:mreal, m:m + 1], lg[:mreal, :], ACTF.Sigmoid, bias=st.c3b[:mreal, :]
    )
    nc.vector.tensor_mul(
        st.o0[:mreal, m:m + 1], st.pred[:mreal, m:m + 1], st.dmin[:mreal, m:m + 1]
    )


def _emit_out_dma(nc, st, out0, out1):
    # native [128, MB] layout; the host flattens p-major and trims to TQ
    n = st.n
    nc.gpsimd.dma_start(out=out1[n], in_=st.pred[:, :])
    nc.gpsimd.dma_start(out=out0[n], in_=st.o0[:, :])


def build_program(per_core=PER_CORE):
    nc = bacc.Bacc("TRN2", target_bir_lowering=False, debug=False)
    x_query = nc.dram_tensor("x_query", [per_core, TQ, D], F32, kind="ExternalInput").ap()
    x_support = nc.dram_tensor("x_support", [per_core, TS, D], F32, kind="ExternalInput").ap()
    x_cls = nc.dram_tensor("x_support_cls", [per_core, S * D], F32, kind="ExternalInput").ap()
    w_head = nc.dram_tensor("W_head", [3 * D, 1], F32, kind="ExternalInput").ap()
    b_head = nc.dram_tensor("b_head", [1, 1], F32, kind="ExternalInput").ap()
    out0 = nc.dram_tensor("out0", [per_core, 128, MB], F32, kind="ExternalOutput").ap()
    out1 = nc.dram_tensor("out1", [per_core, 128, MB], F32, kind="ExternalOutput").ap()
    p2d_list = [nc.dram_tensor(f"p2d_{n}", [TS, 1], F32).ap() for n in range(per_core)]
    c3d_list = [nc.dram_tensor(f"c3d_{n}", [1, 1], F32).ap() for n in range(per_core)]

    with tile.TileContext(nc) as tc, ExitStack() as ctx:
        img_pool = ctx.enter_context(tc.tile_pool(name="img", bufs=2))
        spool = ctx.enter_context(tc.tile_pool(name="sT", bufs=4))
        scratch = ctx.enter_context(tc.tile_pool(name="scratch", bufs=4))
        const_pool = ctx.enter_context(tc.tile_pool(name="const", bufs=1))
        psum_mm = ctx.enter_context(tc.tile_pool(name="psum_mm", bufs=6, space="PSUM"))
        psum_t = ctx.enter_context(tc.tile_pool(name="psum_t", bufs=2, space="PSUM"))

        consts = _emit_consts(nc, const_pool, scratch, w_head, b_head)

        def m_order(j):
            return list(range(MB)) if j == 0 else [MB - 1] + list(range(MB - 1))

        st0 = Image(nc, img_pool, 0)
        st1 = Image(nc, img_pool, 1)
        _emit_cls(nc, st0, scratch, consts, x_cls, c3d_list[0])
        _emit_cls(nc, st1, scratch, consts, x_cls, c3d_list[1])

        # ---- image 0 startup: all loads first (sync queue never blocked),
        # then norms (query chunks front-loaded on scalar), then xposes ----
        _emit_sT_load(nc, st0, scratch, x_support, 0)
        for c in range(3):
            _emit_qT_load(nc, st0, scratch, x_query, c)
        _emit_sT_load(nc, st0, scratch, x_support, 1)
        for c in range(3, 6):
            _emit_qT_load(nc, st0, scratch, x_query, c)
        _emit_sT_load(nc, st0, scratch, x_support, 2)
        _emit_sT_load(nc, st0, scratch, x_support, 3)
        _emit_sT_load(nc, st0, scratch, x_support, 4)
        _emit_sT_norm(nc, st0, scratch, 0, dve_square=True)
        _emit_qT_norm(nc, st0, scratch, 0)
        _emit_qT_norm(nc, st0, scratch, 1)
        _emit_qT_norm(nc, st0, scratch, 2)
        _emit_sT_norm(nc, st0, scratch, 1, dve_square=True)
        _emit_qT_norm(nc, st0, scratch, 3)
        _emit_sT_norm(nc, st0, scratch, 2, dve_square=True)
        sts = {0: _emit_sT_xpose(nc, st0, spool, psum_t, consts, 0)}
        _emit_qT_xpose(nc, st0, psum_t, consts, 0)
        _emit_qT_xpose(nc, st0, psum_t, consts, 1)

        # image 1 qT chunk schedule inside image 0's j-loop
        qt1_load = {1: [0, 1], 2: [2, 3], 3: [4, 5], 4: [6, 7], 5: [8, 9], 6: [10]}
        qt1_norm = {2: [0, 1], 3: [2, 3], 4: [4, 5], 5: [6, 7], 6: [8, 9], 7: [10]}
        qt1_xpose = {3: [0, 1], 4: [2, 3], 5: [4, 5], 6: [6, 7], 7: [8, 9], 8: [10]}
        # image 1 sT pre-priming inside image 0's loop tail
        st1_pre = {6: [(0, None)], 7: [(1, None)], 8: [(2, None)],
                   9: [(3, 0)], 10: [(4, 1)]}

        def emit_loop(st, p2d, qt1l, qt1n, qt1x, head_sched, other, pre_sched):
            for j in range(NB):
                if j + 3 < NB:
                    _emit_sT_norm(nc, st, scratch, j + 3)
                if j + 1 < NB and (j + 1) not in sts:
                    sts[j + 1] = _emit_sT_xpose(nc, st, spool, psum_t, consts, j + 1)
                if j + 5 < NB:
                    _emit_sT_load(nc, st, scratch, x_support, j + 5)
                for c in qt1l.get(j, []):
                    _emit_qT_load(nc, other, scratch, x_query, c)
                for c in qt1n.get(j, []):
                    _emit_qT_norm(nc, other, scratch, c)
                for (lj, nj) in pre_sched.get(j, []):
                    _emit_sT_load(nc, other, scratch, x_support, lj)
                    if nj is not None:
                        _emit_sT_norm(nc, other, scratch, nj, dve_square=True)
                for m in m_order(j):
                    _emit_mm_block(nc, st, psum_mm, scratch, sts[j], j, m, p2d)
                    if st.n == 0 and j == 0 and m + 6 <= MB - 1:
                        _emit_qT_load(nc, st, scratch, x_query, m + 6)
                    if st.n == 0 and j == 0 and m + 4 <= MB - 1:
                        _emit_qT_norm(nc, st, scratch, m + 4)
                    if st.n == 0 and j == 0 and m + 2 <= MB - 1:
                        _emit_qT_xpose(nc, st, psum_t, consts, m + 2)
                    if st.n == 1 and j == NB - 1:
                        _emit_icf_m(nc, st, consts, m)
                        _emit_argmax_m(nc, st, scratch, consts, m)
                        _emit_head_m(nc, st, scratch, m, p2d)
                for c in qt1x.get(j, []):
                    _emit_qT_xpose(nc, other, psum_t, consts, c)
                if head_sched:
                    for m in head_sched.get(j, []):
                        _emit_icf_m(nc, other, consts, m)
                        _emit_argmax_m(nc, other, scratch, consts, m)
                        _emit_head_m(nc, other, scratch, m, p2d_list[0])
                    if j == 7:
                        _emit_out_dma(nc, other, out0, out1)
                del sts[j]

        # ---- image 0 main loop (image 1 qT + sT priming interleaved) ----
        emit_loop(st0, p2d_list[0], qt1_load, qt1_norm, qt1_xpose, None, st1, st1_pre)

        # ---- transition: finish image 1 priming ----
        _emit_sT_norm(nc, st1, scratch, 2, dve_square=True)
        sts = {0: _emit_sT_xpose(nc, st1, spool, psum_t, consts, 0)}

        # image-0 argmax + heads spread over image 1's j-loop iterations
        # (after each j's max/finds so PSUM drains keep priority)
        head0_sched = {1: [0, 1], 2: [2, 3], 3: [4, 5], 4: [6, 7], 5: [8, 9],
                       6: [10]}

        # ---- image 1 main loop ----
        emit_loop(st1, p2d_list[1], {}, {}, {}, head0_sched, st0, {})
        _emit_out_dma(nc, st1, out0, out1)

    nc.compile()
    return nc


_CACHED = {}


def _get_program(per_core=PER_CORE):
    if per_core not in _CACHED:
        _CACHED[per_core] = build_program(per_core)
    return _CACHED[per_core]


def run(inputs, trace=False, per_core=PER_CORE):
    nc = _get_program(per_core)
    n_cores = N_FULL // per_core
    xq = np.ascontiguousarray(inputs["x_query"], dtype=np.float32)
    xs = np.ascontiguousarray(inputs["x_support"], dtype=np.float32)
    xc = np.ascontiguousarray(inputs["x_support_cls"], dtype=np.float32).reshape(
        N_FULL, S * D
    )
    wh = np.ascontiguousarray(inputs["W_head"], dtype=np.float32).reshape(3 * D, 1)
    bhv = np.ascontiguousarray(inputs["b_head"], dtype=np.float32).reshape(1, 1)
    in_maps = []
    for c in range(n_cores):
        sl = slice(c * per_core, (c + 1) * per_core)
        in_maps.append({
            "x_query": xq[sl], "x_support": xs[sl], "x_support_cls": xc[sl],
            "W_head": wh, "b_head": bhv,
        })
    res = run_bass_kernel_spmd(nc, in_maps, list(range(n_cores)), trace=trace)
    o0 = np.concatenate([res.results[c]["out0"] for c in range(n_cores)], axis=0)
    o1 = np.concatenate([res.results[c]["out1"] for c in range(n_cores)], axis=0)
    # [N, 128, MB] (p, m) -> token index m*128+p
    o0 = o0.transpose(0, 2, 1).reshape(N_FULL, 128 * MB)[:, :TQ]
    o1 = o1.transpose(0, 2, 1).reshape(N_FULL, 128 * MB)[:, :TQ]
    o0 = np.ascontiguousarray(o0).reshape(N_FULL, 1, SIDE, SIDE).astype(np.float32)
    o1 = np.ascontiguousarray(o1).reshape(N_FULL, 1, SIDE, SIDE).astype(np.float32)
    return (o0, o1), res


def kernel(**inputs):
    (o0, o1), _ = run(inputs, trace=False)
    return o0, o1
